# revision 2
# baseline (speedup 1.0000x reference)
"""Trainium2 Bass kernel for nn_AdditiveAttention (pooling, memory-bound).

reference:
    weight = scale * W / ||W||                      # [1, S]
    h = tanh(input[:,None,:] + features + bias)     # [B, T, S]
    scores = h @ weight.T                           # [B, T, 1]
    scores = where(mask, scores, -inf)
    wts = softmax(scores, axis=1)
    context = sum_t features * wts                  # [B, S]
    returns (context, wts)

Strategy (8 cores, data-parallel over B=32 -> 4 batches/core):
  - Host: convert features to bf16 in TWO layouts: fT [b, sb, 128s, T]
    (S on partitions -> fused ACT tanh(fT + q_bias) + PE score matvec
    with the 1-column weight as stationary), and fN [b, c, 128t, S]
    (T on partitions -> PE context matvec with softmax weights as
    stationary).  2 x 16MB bf16 reads/core == one 32MB f32 read.
  - |scores| <= ||w||_1 <= 23 so exp() without max-subtraction is safe
    in f32; softmax weights computed unnormalized then scaled by 1/sum.
  - Everything downstream of the bf16 feature streams is f32 (PSUM
    accumulation, exp, normalization).
"""

import os
import sys

import numpy as np

for _p in ("/opt/trn_rl_repo", "/root/.axon_site/_ro/trn_rl_repo"):
    if os.path.isdir(_p) and _p not in sys.path:
        sys.path.append(_p)

import ml_dtypes

BF16 = ml_dtypes.bfloat16

B, T, S = 32, 4096, 512
NCORES = 8
BL = B // NCORES          # batches per core = 4
PB = 128                  # partitions
SB = S // PB              # s-blocks = 4
C = T // PB               # t-chunks = 32
NSC = T // 512            # score supercolumns = 8

_CACHE = {}


def _build_nc():
    """Build + compile the per-core Bass module (identical on all cores)."""
    from contextlib import ExitStack

    from concourse import bacc, mybir, tile

    dt = mybir.dt
    Act = mybir.ActivationFunctionType
    Alu = mybir.AluOpType
    Ax = mybir.AxisListType

    nc = bacc.Bacc(
        "TRN2",
        target_bir_lowering=False,
        debug=False,
        enable_asserts=False,
        num_devices=NCORES,
    )

    ft_d = nc.dram_tensor("ft", [BL, SB, PB, T], dt.bfloat16, kind="ExternalInput").ap()
    fn_d = nc.dram_tensor("fn", [BL, C, PB, S], dt.bfloat16, kind="ExternalInput").ap()
    qb_d = nc.dram_tensor("qb", [SB, PB, BL], dt.float32, kind="ExternalInput").ap()
    wq_d = nc.dram_tensor("wq", [SB, PB, 1], dt.bfloat16, kind="ExternalInput").ap()
    mk_d = nc.dram_tensor("mk", [BL, 1, T], dt.float32, kind="ExternalInput").ap()
    ctx_d = nc.dram_tensor("ctx", [BL, 1, S], dt.float32, kind="ExternalOutput").ap()
    wts_d = nc.dram_tensor("wts", [BL, 1, T], dt.float32, kind="ExternalOutput").ap()

    with ExitStack() as ctx:
        tc = ctx.enter_context(tile.TileContext(nc))
        singles = ctx.enter_context(tc.tile_pool(name="singles", bufs=1))
        dbl = ctx.enter_context(tc.tile_pool(name="dbl", bufs=2))
        tri = ctx.enter_context(tc.tile_pool(name="tri", bufs=3))
        ps_sg = ctx.enter_context(tc.tile_pool(name="ps_sg", bufs=6, space="PSUM"))
        ps_cx = ctx.enter_context(tc.tile_pool(name="ps_cx", bufs=2, space="PSUM"))

        # replicated params, loaded once
        qt = []
        wqt = []
        for sb in range(SB):
            q_t = singles.tile([PB, BL], dt.float32, tag=f"q{sb}")
            nc.sync.dma_start(out=q_t[:, :], in_=qb_d[sb])
            qt.append(q_t)
            w_t = singles.tile([PB, 1], dt.bfloat16, tag=f"w{sb}")
            nc.sync.dma_start(out=w_t[:, :], in_=wq_d[sb])
            wqt.append(w_t)

        hts = {}     # b -> [ht_sb0..3]
        fnts = {}    # b -> (half0, half1)
        wtbs = {}    # b -> bf16 [128, 32] context stationary

        def emit_loads_and_tanh(b):
            # normal-layout prefetch for the context phase (2 x 2MB)
            halves = []
            for hh in range(2):
                fnt = tri.tile([PB, C // 2, S], dt.bfloat16, tag="fnh")
                src = fn_d[b].transpose([1, 0, 2])  # [C, PB, S] -> [PB, C, S]
                nc.sync.dma_start(
                    out=fnt[:, :, :], in_=src[:, hh * (C // 2):(hh + 1) * (C // 2), :]
                )
                halves.append(fnt)
            fnts[b] = halves
            # transposed stream + fused add+tanh
            hts[b] = []
            for sb in range(SB):
                ft_t = tri.tile([PB, T], dt.bfloat16, tag="ft")
                nc.sync.dma_start(out=ft_t[:, :], in_=ft_d[b, sb])
                ht_t = dbl.tile([PB, T], dt.bfloat16, tag=f"ht{sb}")
                nc.scalar.activation(
                    ht_t[:, :], ft_t[:, :], Act.Tanh,
                    bias=qt[sb][:, b:b + 1], scale=1.0,
                )
                hts[b].append(ht_t)

        def emit_scores_softmax(b):
            u_row = singles.tile([1, T], dt.float32, tag="u")
            for sc in range(NSC):
                sg = ps_sg.tile([1, 512], dt.float32, tag="sg")
                for sb in range(SB):
                    nc.tensor.matmul(
                        sg[:, :],
                        wqt[sb][:, :],
                        hts[b][sb][:, sc * 512:(sc + 1) * 512],
                        start=(sb == 0),
                        stop=(sb == SB - 1),
                    )
                nc.scalar.activation(
                    u_row[:, sc * 512:(sc + 1) * 512], sg[:, :], Act.Exp
                )
            # mask (all-ones in practice; exact: u *= mask == exp(-inf)=0)
            m_row = singles.tile([1, T], dt.float32, tag="m")
            nc.sync.dma_start(out=m_row[:, :], in_=mk_d[b])
            nc.vector.tensor_mul(u_row[:, :], u_row[:, :], m_row[:, :])
            du = singles.tile([1, 1], dt.float32, tag="du")
            nc.vector.tensor_reduce(du[:, :], u_row[:, :], axis=Ax.X, op=Alu.add)
            rc = singles.tile([1, 1], dt.float32, tag="rc")
            nc.vector.reciprocal(rc[:, :], du[:, :])
            wt_row = singles.tile([1, T], dt.float32, tag="wt")
            nc.vector.tensor_scalar_mul(wt_row[:, :], u_row[:, :], rc[:, :])
            # weights output, plus DRAM round-trip to re-block [1,T] -> [128,C]
            nc.sync.dma_start(out=wts_d[b], in_=wt_row[:, :])
            wtb32 = dbl.tile([PB, C], dt.float32, tag="wtb32")
            nc.sync.dma_start(
                out=wtb32[:, :],
                in_=wts_d[b].rearrange("o (c p) -> (o p) c", p=PB),
            )
            wtb = dbl.tile([PB, C], dt.bfloat16, tag="wtb")
            nc.vector.tensor_copy(wtb[:, :], wtb32[:, :])
            wtbs[b] = wtb

        def emit_context(b):
            cx = ps_cx.tile([1, S], dt.float32, tag="cx")
            wtb = wtbs[b]
            for cc in range(C):
                hh, cl = divmod(cc, C // 2)
                nc.tensor.matmul(
                    cx[:, :],
                    wtb[:, cc:cc + 1],
                    fnts[b][hh][:, cl, :],
                    start=(cc == 0),
                    stop=(cc == C - 1),
                )
            cs = dbl.tile([1, S], dt.float32, tag="cs")
            nc.vector.tensor_copy(cs[:, :], cx[:, :])
            nc.sync.dma_start(out=ctx_d[b], in_=cs[:, :])

        for b in range(BL):
            emit_loads_and_tanh(b)
            if b > 0:
                emit_context(b - 1)          # PE: ctx(b-1) runs while tanh(b) streams
            emit_scores_softmax(b)
        emit_context(BL - 1)

    nc.compile()
    return nc


def _make_runner(nc):
    """Cached shard_map executor over the 8 axon NeuronCores (mirrors
    run_bass_via_pjrt's multi-core branch so repeat calls reuse the jit)."""
    import jax
    from jax.experimental.shard_map import shard_map
    from jax.sharding import Mesh, PartitionSpec

    from concourse import mybir
    from concourse.bass2jax import (
        _bass_exec_p,
        install_neuronx_cc_hook,
        partition_id_tensor,
    )

    install_neuronx_cc_hook()

    partition_name = nc.partition_id_tensor.name if nc.partition_id_tensor else None
    in_names, out_names, out_avals, zero_shapes = [], [], [], []
    for alloc in nc.m.functions[0].allocations:
        if not isinstance(alloc, mybir.MemoryLocationSet):
            continue
        name = alloc.memorylocations[0].name
        if alloc.kind == "ExternalInput":
            if name != partition_name:
                in_names.append(name)
        elif alloc.kind == "ExternalOutput":
            out_names.append(name)
            shape = tuple(alloc.tensor_shape)
            dtype = mybir.dt.np(alloc.dtype)
            out_avals.append(jax.core.ShapedArray(shape, dtype))
            zero_shapes.append((shape, dtype))
    n_params = len(in_names)
    n_outs = len(out_names)
    all_names = in_names + out_names
    if partition_name is not None:
        all_names = all_names + [partition_name]
    donate = tuple(range(n_params, n_params + n_outs))

    def _body(*args):
        operands = list(args)
        if partition_name is not None:
            operands.append(partition_id_tensor())
        outs = _bass_exec_p.bind(
            *operands,
            out_avals=tuple(out_avals),
            in_names=tuple(all_names),
            out_names=tuple(out_names),
            lowering_input_output_aliases=(),
            sim_require_finite=True,
            sim_require_nnan=True,
            nc=nc,
        )
        return tuple(outs)

    devices = jax.devices()[:NCORES]
    mesh = Mesh(np.asarray(devices), ("core",))
    in_specs = (PartitionSpec("core"),) * (n_params + n_outs)
    out_specs = (PartitionSpec("core"),) * n_outs
    sharded = jax.jit(
        shard_map(_body, mesh=mesh, in_specs=in_specs, out_specs=out_specs,
                  check_rep=False),
        donate_argnums=donate,
        keep_unused=True,
    )

    def run(in_maps):
        concat_in = [
            np.concatenate([np.asarray(m[name]) for m in in_maps], axis=0)
            for name in in_names
        ]
        concat_zeros = [
            np.zeros((NCORES * s[0], *s[1:]), d) for (s, d) in zero_shapes
        ]
        out_arrs = sharded(*concat_in, *concat_zeros)
        out_np = [np.asarray(a) for a in out_arrs]
        return [
            {
                name: out_np[i].reshape(NCORES, *out_avals[i].shape)[c]
                for i, name in enumerate(out_names)
            }
            for c in range(NCORES)
        ]

    return run


def get_runner():
    if "runner" not in _CACHE:
        nc = _build_nc()
        _CACHE["nc"] = nc
        _CACHE["runner"] = _make_runner(nc)
    return _CACHE["runner"]


def prep(input, features, features_mask, W, scale, attn_bias):
    """Host-side shard + layout prep -> list of 8 per-core input dicts."""
    input = np.asarray(input, np.float32)
    features = np.asarray(features, np.float32)
    mask = np.asarray(features_mask).reshape(B, T)
    W = np.asarray(W, np.float32)
    scale = np.asarray(scale, np.float32)
    attn_bias = np.asarray(attn_bias, np.float32)

    w = (scale.reshape(()) * W[0] / np.linalg.norm(W[0]))  # [S] f32
    wq = np.ascontiguousarray(w.astype(BF16).reshape(SB, PB, 1))
    q = input + attn_bias[None, :]                          # [B, S] f32
    mk = mask.astype(np.float32).reshape(B, 1, T)

    f_bf = features.astype(BF16)                            # [B, T, S]

    in_maps = []
    for i in range(NCORES):
        bs = slice(i * BL, (i + 1) * BL)
        fc = f_bf[bs]                                       # [BL, T, S]
        ft = np.ascontiguousarray(fc.transpose(0, 2, 1)).reshape(BL, SB, PB, T)
        fn = fc.reshape(BL, C, PB, S)
        qb = np.ascontiguousarray(
            q[bs].reshape(BL, SB, PB).transpose(1, 2, 0))   # [SB, PB, BL]
        in_maps.append({
            "ft": np.ascontiguousarray(ft),
            "fn": np.ascontiguousarray(fn),
            "qb": qb,
            "wq": wq,
            "mk": np.ascontiguousarray(mk[bs]),
        })
    return in_maps


def run_device(in_maps):
    results = get_runner()(in_maps)
    context = np.concatenate(
        [r["ctx"].reshape(BL, S) for r in results], axis=0).astype(np.float32)
    weights = np.concatenate(
        [r["wts"].reshape(BL, T) for r in results], axis=0).astype(np.float32)
    return context, weights[:, :, None]


def kernel(input, features, features_mask, W, scale, attn_bias):
    in_maps = prep(input, features, features_mask, W, scale, attn_bias)
    context, weights = run_device(in_maps)
    return (context, weights)


# revision 6
# speedup vs baseline: 1.3498x; 1.3498x over previous
"""Trainium2 Bass kernel for nn_AdditiveAttention (pooling, memory-bound).

reference:
    weight = scale * W / ||W||                      # [1, S]
    h = tanh(input[:,None,:] + features + bias)     # [B, T, S]
    scores = h @ weight.T                           # [B, T, 1]
    scores = where(mask, scores, -inf)
    wts = softmax(scores, axis=1)
    context = sum_t features * wts                  # [B, S]
    returns (context, wts)

Strategy (8 cores, data-parallel over B=32 -> 4 batches/core):
  - Host: features -> bf16 in TWO layouts: ft [b, sb, 128s, T] (S on
    partitions -> fused ACT tanh(ft + q_bias), then PE score matvec with
    the 1-column normalized weight as stationary, accumulated in PSUM),
    and fn [b, 128p, 32c, S] (T on partitions, t = p*32+c, a pure
    reshape -> PE context matvec).  2 x 16MB bf16 reads/core == one 32MB
    f32 read.
  - |scores| <= ||w||_1 <= 23 so exp() without max-subtraction is safe in
    f32. The context matvec uses UNNORMALIZED u = exp(scores) (bf16) as
    stationary; 1/sum(u) is applied to the [1,S] PSUM result at evac.
    sum(u) itself is a PE ones-matvec over the re-blocked u.
  - u is re-blocked [1,T] -> [128,32] via a DRAM round-trip whose read
    is 128 contiguous 128B runs (t = p*32+c layout).
  - The all-ones mask (the harness case) is specialized away at host
    level; a general-mask variant kernel is compiled only if needed.
"""

import os
import sys

import numpy as np

for _p in ("/opt/trn_rl_repo", "/root/.axon_site/_ro/trn_rl_repo"):
    if os.path.isdir(_p) and _p not in sys.path:
        sys.path.append(_p)

import ml_dtypes

BF16 = ml_dtypes.bfloat16

B, T, S = 32, 4096, 512
NCORES = 8
BL = B // NCORES          # batches per core = 4
PB = 128                  # partitions
SB = S // PB              # s-blocks = 4
C = T // PB               # t-chunks = 32
NSC = T // 512            # score supercolumns = 8

_CACHE = {}


def _build_nc(with_mask=False):
    """Build + compile the per-core Bass module (identical on all cores)."""
    from contextlib import ExitStack

    from concourse import bacc, mybir, tile

    dt = mybir.dt
    Act = mybir.ActivationFunctionType
    Alu = mybir.AluOpType
    Ax = mybir.AxisListType

    nc = bacc.Bacc(
        "TRN2",
        target_bir_lowering=False,
        debug=False,
        enable_asserts=False,
        num_devices=NCORES,
    )

    ft_d = nc.dram_tensor("ft", [BL, SB, PB, T], dt.bfloat16, kind="ExternalInput").ap()
    # partition-major: fn[b, p, c, s] = features[b, p*C + c, s] (pure reshape)
    fn_d = nc.dram_tensor("fn", [BL, PB, C, S], dt.bfloat16, kind="ExternalInput").ap()
    qb_d = nc.dram_tensor("qb", [SB, PB, BL], dt.float32, kind="ExternalInput").ap()
    wq_d = nc.dram_tensor("wq", [SB, PB, 1], dt.bfloat16, kind="ExternalInput").ap()
    if with_mask:
        mk_d = nc.dram_tensor("mk", [BL, 1, T], dt.float32, kind="ExternalInput").ap()
    ctx_d = nc.dram_tensor("ctx", [BL, 1, S], dt.float32, kind="ExternalOutput").ap()
    wts_d = nc.dram_tensor("wts", [BL, 1, T], dt.float32, kind="ExternalOutput").ap()
    scr_d = nc.dram_tensor("scr", [BL, 1, T], dt.float32).ap()  # u round-trip

    with ExitStack() as ctx:
        tc = ctx.enter_context(tile.TileContext(nc))
        singles = ctx.enter_context(tc.tile_pool(name="singles", bufs=1))
        dbl = ctx.enter_context(tc.tile_pool(name="dbl", bufs=2))
        tri = ctx.enter_context(tc.tile_pool(name="tri", bufs=3))
        ps_sg = ctx.enter_context(tc.tile_pool(name="ps_sg", bufs=5, space="PSUM"))
        ps_cx = ctx.enter_context(tc.tile_pool(name="ps_cx", bufs=2, space="PSUM"))
        ps_su = ctx.enter_context(tc.tile_pool(name="ps_su", bufs=1, space="PSUM"))

        # replicated params, loaded once
        qt, wqt = [], []
        for sb in range(SB):
            q_t = singles.tile([PB, BL], dt.float32, tag=f"q{sb}")
            nc.sync.dma_start(out=q_t[:, :], in_=qb_d[sb])
            qt.append(q_t)
            w_t = singles.tile([PB, 1], dt.bfloat16, tag=f"w{sb}")
            nc.sync.dma_start(out=w_t[:, :], in_=wq_d[sb])
            wqt.append(w_t)
        ones = singles.tile([PB, 1], dt.float32, tag="ones")
        nc.vector.memset(ones[:, :], 1.0)

        hts = {}     # b -> [ht_sb0..3]
        fnts = {}    # b -> (half0, half1)
        ubfs = {}    # b -> bf16 [128, 32] unnormalized-u stationary
        ublks = {}   # b -> f32 [128, 32] for the sum
        u_rows = {}  # b -> f32 [1, T] unnormalized (masked) u

        def emit_loads_and_tanh(b):
            # transposed stream + fused add+tanh (spine head: issue first)
            hts[b] = []
            for sb in range(SB):
                ft_t = tri.tile([PB, T], dt.bfloat16, tag="ft")
                nc.sync.dma_start(out=ft_t[:, :], in_=ft_d[b, sb])
                ht_t = dbl.tile([PB, T], dt.bfloat16, tag=f"ht{sb}")
                nc.scalar.activation(
                    ht_t[:, :], ft_t[:, :], Act.Tanh,
                    bias=qt[sb][:, b:b + 1], scale=1.0,
                )
                hts[b].append(ht_t)
            # normal-layout prefetch for the context phase (2 x 2MB)
            halves = []
            for hh in range(2):
                fnt = tri.tile([PB, C // 2, S], dt.bfloat16, tag="fnh")
                nc.sync.dma_start(
                    out=fnt[:, :, :],
                    in_=fn_d[b][:, hh * (C // 2):(hh + 1) * (C // 2), :]
                )
                halves.append(fnt)
            fnts[b] = halves

        def emit_scores_softmax(b):
            u_row = dbl.tile([1, T], dt.float32, tag="u")
            for sc in range(NSC):
                sg = ps_sg.tile([1, 512], dt.float32, tag="sg")
                for sb in range(SB):
                    nc.tensor.matmul(
                        sg[:, :],
                        wqt[sb][:, :],
                        hts[b][sb][:, sc * 512:(sc + 1) * 512],
                        start=(sb == 0),
                        stop=(sb == SB - 1),
                    )
                nc.scalar.activation(
                    u_row[:, sc * 512:(sc + 1) * 512], sg[:, :], Act.Exp
                )
            if with_mask:
                m_row = dbl.tile([1, T], dt.float32, tag="m")
                nc.sync.dma_start(out=m_row[:, :], in_=mk_d[b])
                um_row = dbl.tile([1, T], dt.float32, tag="um")
                nc.vector.tensor_mul(um_row[:, :], u_row[:, :], m_row[:, :])
                u_row = um_row
            u_rows[b] = u_row
            # unnormalized u out + re-block readback [1,T] -> [128,C]
            nc.sync.dma_start(out=scr_d[b], in_=u_row[:, :])
            ublk = dbl.tile([PB, C], dt.float32, tag="ublk")
            nc.sync.dma_start(
                out=ublk[:, :],
                in_=scr_d[b].rearrange("o (p c) -> (o p) c", c=C),
            )
            u_bf = dbl.tile([PB, C], dt.bfloat16, tag="ubf")
            nc.vector.tensor_copy(u_bf[:, :], ublk[:, :])
            ubfs[b] = u_bf
            ublks[b] = ublk

        def emit_context(b):
            # sum(u): ones^T @ ublk -> psum [1, C] -> reduce -> reciprocal
            su = ps_su.tile([1, C], dt.float32, tag="su")
            nc.tensor.matmul(su[:, :], ones[:, :], ublks[b][:, :],
                             start=True, stop=True)
            du = singles.tile([1, 1], dt.float32, tag="du")
            nc.vector.tensor_reduce(du[:, :], su[:, :], axis=Ax.X, op=Alu.add)
            rc = singles.tile([1, 1], dt.float32, tag=f"rc{b}")
            nc.vector.reciprocal(rc[:, :], du[:, :])
            # context matvec with unnormalized u, scaled at evacuation
            cx = ps_cx.tile([1, S], dt.float32, tag="cx")
            for cc in range(C):
                hh, cl = divmod(cc, C // 2)
                nc.tensor.matmul(
                    cx[:, :],
                    ubfs[b][:, cc:cc + 1],
                    fnts[b][hh][:, cl, :],
                    start=(cc == 0),
                    stop=(cc == C - 1),
                )
            cs = dbl.tile([1, S], dt.float32, tag="cs")
            nc.vector.tensor_scalar_mul(cs[:, :], cx[:, :], rc[:, :])
            nc.sync.dma_start(out=ctx_d[b], in_=cs[:, :])
            # normalized weights output (off the context critical path)
            wt_row = singles.tile([1, T], dt.float32, tag="wt")
            nc.vector.tensor_scalar_mul(wt_row[:, :], u_rows[b][:, :], rc[:, :])
            nc.sync.dma_start(out=wts_d[b], in_=wt_row[:, :])

        for b in range(BL):
            emit_loads_and_tanh(b)
            if b > 0:
                emit_context(b - 1)          # PE: ctx(b-1) runs while tanh(b) streams
            emit_scores_softmax(b)
        emit_context(BL - 1)

    nc.compile()
    return nc


def _make_runner(nc):
    """Cached shard_map executor over the 8 axon NeuronCores (mirrors
    run_bass_via_pjrt's multi-core branch so repeat calls reuse the jit)."""
    import jax
    from jax.experimental.shard_map import shard_map
    from jax.sharding import Mesh, PartitionSpec

    from concourse import mybir
    from concourse.bass2jax import (
        _bass_exec_p,
        install_neuronx_cc_hook,
        partition_id_tensor,
    )

    install_neuronx_cc_hook()

    partition_name = nc.partition_id_tensor.name if nc.partition_id_tensor else None
    in_names, out_names, out_avals, zero_shapes = [], [], [], []
    for alloc in nc.m.functions[0].allocations:
        if not isinstance(alloc, mybir.MemoryLocationSet):
            continue
        name = alloc.memorylocations[0].name
        if alloc.kind == "ExternalInput":
            if name != partition_name:
                in_names.append(name)
        elif alloc.kind == "ExternalOutput":
            out_names.append(name)
            shape = tuple(alloc.tensor_shape)
            dtype = mybir.dt.np(alloc.dtype)
            out_avals.append(jax.core.ShapedArray(shape, dtype))
            zero_shapes.append((shape, dtype))
    n_params = len(in_names)
    n_outs = len(out_names)
    all_names = in_names + out_names
    if partition_name is not None:
        all_names = all_names + [partition_name]
    donate = tuple(range(n_params, n_params + n_outs))

    def _body(*args):
        operands = list(args)
        if partition_name is not None:
            operands.append(partition_id_tensor())
        outs = _bass_exec_p.bind(
            *operands,
            out_avals=tuple(out_avals),
            in_names=tuple(all_names),
            out_names=tuple(out_names),
            lowering_input_output_aliases=(),
            sim_require_finite=True,
            sim_require_nnan=True,
            nc=nc,
        )
        return tuple(outs)

    devices = jax.devices()[:NCORES]
    mesh = Mesh(np.asarray(devices), ("core",))
    in_specs = (PartitionSpec("core"),) * (n_params + n_outs)
    out_specs = (PartitionSpec("core"),) * n_outs
    sharded = jax.jit(
        shard_map(_body, mesh=mesh, in_specs=in_specs, out_specs=out_specs,
                  check_rep=False),
        donate_argnums=donate,
        keep_unused=True,
    )

    def run(in_maps):
        concat_in = [
            np.concatenate([np.asarray(m[name]) for m in in_maps], axis=0)
            for name in in_names
        ]
        concat_zeros = [
            np.zeros((NCORES * s[0], *s[1:]), d) for (s, d) in zero_shapes
        ]
        out_arrs = sharded(*concat_in, *concat_zeros)
        out_np = [np.asarray(a) for a in out_arrs]
        return [
            {
                name: out_np[i].reshape(NCORES, *out_avals[i].shape)[c]
                for i, name in enumerate(out_names)
            }
            for c in range(NCORES)
        ]

    return run


def get_runner(with_mask=False):
    key = ("runner", with_mask)
    if key not in _CACHE:
        nc = _build_nc(with_mask=with_mask)
        _CACHE[("nc", with_mask)] = nc
        _CACHE[key] = _make_runner(nc)
    return _CACHE[key]


def prep(input, features, features_mask, W, scale, attn_bias):
    """Host-side shard + layout prep -> (list of 8 per-core dicts, with_mask)."""
    input = np.asarray(input, np.float32)
    features = np.asarray(features, np.float32)
    mask = np.asarray(features_mask).reshape(B, T)
    W = np.asarray(W, np.float32)
    scale = np.asarray(scale, np.float32)
    attn_bias = np.asarray(attn_bias, np.float32)

    with_mask = not bool(mask.all())
    w = (scale.reshape(()) * W[0] / np.linalg.norm(W[0]))  # [S] f32
    wq = np.ascontiguousarray(w.astype(BF16).reshape(SB, PB, 1))
    q = input + attn_bias[None, :]                          # [B, S] f32

    f_bf = features.astype(BF16)                            # [B, T, S]

    in_maps = []
    for i in range(NCORES):
        bs = slice(i * BL, (i + 1) * BL)
        fc = f_bf[bs]                                       # [BL, T, S]
        ft = np.ascontiguousarray(fc.transpose(0, 2, 1)).reshape(BL, SB, PB, T)
        fn = fc.reshape(BL, PB, C, S)
        qb = np.ascontiguousarray(
            q[bs].reshape(BL, SB, PB).transpose(1, 2, 0))   # [SB, PB, BL]
        m = {
            "ft": np.ascontiguousarray(ft),
            "fn": np.ascontiguousarray(fn),
            "qb": qb,
            "wq": wq,
        }
        if with_mask:
            m["mk"] = np.ascontiguousarray(
                mask[bs].astype(np.float32).reshape(BL, 1, T))
        in_maps.append(m)
    return in_maps, with_mask


def run_device(in_maps, with_mask=False):
    results = get_runner(with_mask=with_mask)(in_maps)
    context = np.concatenate(
        [r["ctx"].reshape(BL, S) for r in results], axis=0).astype(np.float32)
    weights = np.concatenate(
        [r["wts"].reshape(BL, T) for r in results], axis=0).astype(np.float32)
    return context, weights[:, :, None]


def kernel(input, features, features_mask, W, scale, attn_bias):
    in_maps, with_mask = prep(input, features, features_mask, W, scale, attn_bias)
    context, weights = run_device(in_maps, with_mask=with_mask)
    return (context, weights)


# revision 9
# speedup vs baseline: 1.3765x; 1.0197x over previous
"""Trainium2 Bass kernel for nn_AdditiveAttention (pooling, memory-bound).

reference:
    weight = scale * W / ||W||                      # [1, S]
    h = tanh(input[:,None,:] + features + bias)     # [B, T, S]
    scores = h @ weight.T                           # [B, T, 1]
    scores = where(mask, scores, -inf)
    wts = softmax(scores, axis=1)
    context = sum_t features * wts                  # [B, S]
    returns (context, wts)

Strategy (8 cores, data-parallel over B=32 -> 4 batches/core):
  - Host: features -> bf16 in TWO layouts: ft [b, sb, 128s, T] (S on
    partitions -> fused ACT tanh(ft + q_bias), then PE score matvec with
    the 1-column normalized weight as stationary, accumulated in PSUM),
    and fn [b, 128p, 32c, S] (T on partitions, t = p*32+c, a pure
    reshape -> PE context matvec).  2 x 16MB bf16 reads/core == one 32MB
    f32 read.
  - |scores| <= ||w||_1 <= 23 so exp() without max-subtraction is safe in
    f32. The context matvec uses UNNORMALIZED u = exp(scores) (bf16) as
    stationary; 1/sum(u) is applied to the [1,S] PSUM result at evac.
    sum(u) itself is a PE ones-matvec over the re-blocked u.
  - u is re-blocked [1,T] -> [128,32] via a DRAM round-trip whose read
    is 128 contiguous 128B runs (t = p*32+c layout).
  - The all-ones mask (the harness case) is specialized away at host
    level; a general-mask variant kernel is compiled only if needed.
"""

import os
import sys

import numpy as np

for _p in ("/opt/trn_rl_repo", "/root/.axon_site/_ro/trn_rl_repo"):
    if os.path.isdir(_p) and _p not in sys.path:
        sys.path.append(_p)

import ml_dtypes

BF16 = ml_dtypes.bfloat16

B, T, S = 32, 4096, 512
NCORES = 8
BL = B // NCORES          # batches per core = 4
PB = 128                  # partitions
SB = S // PB              # s-blocks = 4
C = T // PB               # t-chunks = 32
NSC = T // 512            # score supercolumns = 8

_CACHE = {}


def _build_nc(with_mask=False):
    """Build + compile the per-core Bass module (identical on all cores)."""
    from contextlib import ExitStack

    from concourse import bacc, mybir, tile

    dt = mybir.dt
    Act = mybir.ActivationFunctionType
    Alu = mybir.AluOpType
    Ax = mybir.AxisListType

    nc = bacc.Bacc(
        "TRN2",
        target_bir_lowering=False,
        debug=False,
        enable_asserts=False,
        num_devices=NCORES,
    )

    ft_d = nc.dram_tensor("ft", [BL, SB, PB, T], dt.bfloat16, kind="ExternalInput").ap()
    # partition-major: fn[b, p, c, s] = features[b, p*C + c, s] (pure reshape)
    fn_d = nc.dram_tensor("fn", [BL, PB, C, S], dt.bfloat16, kind="ExternalInput").ap()
    qb_d = nc.dram_tensor("qb", [PB, SB * BL], dt.float32, kind="ExternalInput").ap()
    wq_d = nc.dram_tensor("wq", [PB, SB], dt.bfloat16, kind="ExternalInput").ap()
    if with_mask:
        mk_d = nc.dram_tensor("mk", [BL, 1, T], dt.float32, kind="ExternalInput").ap()
    ctx_d = nc.dram_tensor("ctx", [BL, 1, S], dt.float32, kind="ExternalOutput").ap()
    wts_d = nc.dram_tensor("wts", [BL, 1, T], dt.float32, kind="ExternalOutput").ap()
    scr_d = nc.dram_tensor("scr", [BL, 1, T], dt.float32).ap()  # u round-trip

    with ExitStack() as ctx:
        tc = ctx.enter_context(tile.TileContext(nc))
        singles = ctx.enter_context(tc.tile_pool(name="singles", bufs=1))
        dbl = ctx.enter_context(tc.tile_pool(name="dbl", bufs=2))
        tri = ctx.enter_context(tc.tile_pool(name="tri", bufs=3))
        ps_sg = ctx.enter_context(tc.tile_pool(name="ps_sg", bufs=5, space="PSUM"))
        ps_cx = ctx.enter_context(tc.tile_pool(name="ps_cx", bufs=2, space="PSUM"))
        ps_su = ctx.enter_context(tc.tile_pool(name="ps_su", bufs=1, space="PSUM"))

        # replicated params, loaded once (2 batched DMAs to keep them off
        # the head of the DMA queue)
        qall = singles.tile([PB, SB * BL], dt.float32, tag="qall")
        nc.sync.dma_start(out=qall[:, :], in_=qb_d)
        wall = singles.tile([PB, SB], dt.bfloat16, tag="wall")
        nc.sync.dma_start(out=wall[:, :], in_=wq_d)
        qt = [qall[:, sb * BL:(sb + 1) * BL] for sb in range(SB)]
        wqt = [wall[:, sb:sb + 1] for sb in range(SB)]
        ones = singles.tile([PB, 1], dt.float32, tag="ones")
        nc.vector.memset(ones[:, :], 1.0)

        hts = {}     # b -> [ht_sb0..3]
        fnts = {}    # b -> (half0, half1)
        ubfs = {}    # b -> bf16 [128, 32] unnormalized-u stationary
        ublks = {}   # b -> f32 [128, 32] for the sum
        u_rows = {}  # b -> f32 [1, T] unnormalized (masked) u

        def emit_loads_and_tanh(b):
            # transposed stream + fused add+tanh (spine head: issue first)
            hts[b] = []
            for sb in range(SB):
                ft_t = tri.tile([PB, T], dt.bfloat16, tag="ft")
                nc.sync.dma_start(out=ft_t[:, :], in_=ft_d[b, sb])
                ht_t = dbl.tile([PB, T], dt.bfloat16, tag=f"ht{sb}")
                nc.scalar.activation(
                    ht_t[:, :], ft_t[:, :], Act.Tanh,
                    bias=qt[sb][:, b:b + 1], scale=1.0,
                )
                hts[b].append(ht_t)
            # normal-layout prefetch for the context phase (2 x 2MB)
            halves = []
            for hh in range(2):
                fnt = tri.tile([PB, C // 2, S], dt.bfloat16, tag="fnh")
                nc.sync.dma_start(
                    out=fnt[:, :, :],
                    in_=fn_d[b][:, hh * (C // 2):(hh + 1) * (C // 2), :]
                )
                halves.append(fnt)
            fnts[b] = halves

        def emit_scores_softmax(b):
            # sigma -> SBUF via the (idle) DVE, then ONE exp per batch on ACT
            s_row = singles.tile([1, T], dt.float32, tag="s")
            u_row = dbl.tile([1, T], dt.float32, tag="u")
            for sc in range(NSC):
                sg = ps_sg.tile([1, 512], dt.float32, tag="sg")
                for sb in range(SB):
                    nc.tensor.matmul(
                        sg[:, :],
                        wqt[sb],
                        hts[b][sb][:, sc * 512:(sc + 1) * 512],
                        start=(sb == 0),
                        stop=(sb == SB - 1),
                    )
                nc.vector.tensor_copy(
                    s_row[:, sc * 512:(sc + 1) * 512], sg[:, :]
                )
            nc.scalar.activation(u_row[:, :], s_row[:, :], Act.Exp)
            if with_mask:
                m_row = dbl.tile([1, T], dt.float32, tag="m")
                nc.sync.dma_start(out=m_row[:, :], in_=mk_d[b])
                um_row = dbl.tile([1, T], dt.float32, tag="um")
                nc.vector.tensor_mul(um_row[:, :], u_row[:, :], m_row[:, :])
                u_row = um_row
            u_rows[b] = u_row
            # unnormalized u out + re-block readback [1,T] -> [128,C]
            nc.sync.dma_start(out=scr_d[b], in_=u_row[:, :])
            ublk = dbl.tile([PB, C], dt.float32, tag="ublk")
            nc.sync.dma_start(
                out=ublk[:, :],
                in_=scr_d[b].rearrange("o (p c) -> (o p) c", c=C),
            )
            u_bf = dbl.tile([PB, C], dt.bfloat16, tag="ubf")
            nc.vector.tensor_copy(u_bf[:, :], ublk[:, :])
            ubfs[b] = u_bf
            ublks[b] = ublk

        def emit_context(b):
            # sum(u): ones^T @ ublk -> psum [1, C] -> reduce -> reciprocal
            su = ps_su.tile([1, C], dt.float32, tag="su")
            nc.tensor.matmul(su[:, :], ones[:, :], ublks[b][:, :],
                             start=True, stop=True)
            du = singles.tile([1, 1], dt.float32, tag="du")
            nc.vector.tensor_reduce(du[:, :], su[:, :], axis=Ax.X, op=Alu.add)
            rc = singles.tile([1, 1], dt.float32, tag=f"rc{b}")
            nc.vector.reciprocal(rc[:, :], du[:, :])
            # context matvec with unnormalized u, scaled at evacuation
            cx = ps_cx.tile([1, S], dt.float32, tag="cx")
            for cc in range(C):
                hh, cl = divmod(cc, C // 2)
                nc.tensor.matmul(
                    cx[:, :],
                    ubfs[b][:, cc:cc + 1],
                    fnts[b][hh][:, cl, :],
                    start=(cc == 0),
                    stop=(cc == C - 1),
                )
            cs = dbl.tile([1, S], dt.float32, tag="cs")
            nc.vector.tensor_scalar_mul(cs[:, :], cx[:, :], rc[:, :])
            nc.sync.dma_start(out=ctx_d[b], in_=cs[:, :])
            # normalized weights output (off the context critical path)
            wt_row = singles.tile([1, T], dt.float32, tag="wt")
            nc.vector.tensor_scalar_mul(wt_row[:, :], u_rows[b][:, :], rc[:, :])
            nc.sync.dma_start(out=wts_d[b], in_=wt_row[:, :])

        for b in range(BL):
            emit_loads_and_tanh(b)
            if b > 0:
                emit_context(b - 1)          # PE: ctx(b-1) runs while tanh(b) streams
            emit_scores_softmax(b)
        emit_context(BL - 1)

    nc.compile()
    return nc


def _make_runner(nc):
    """Cached shard_map executor over the 8 axon NeuronCores (mirrors
    run_bass_via_pjrt's multi-core branch so repeat calls reuse the jit)."""
    import jax
    from jax.experimental.shard_map import shard_map
    from jax.sharding import Mesh, PartitionSpec

    from concourse import mybir
    from concourse.bass2jax import (
        _bass_exec_p,
        install_neuronx_cc_hook,
        partition_id_tensor,
    )

    install_neuronx_cc_hook()

    partition_name = nc.partition_id_tensor.name if nc.partition_id_tensor else None
    in_names, out_names, out_avals, zero_shapes = [], [], [], []
    for alloc in nc.m.functions[0].allocations:
        if not isinstance(alloc, mybir.MemoryLocationSet):
            continue
        name = alloc.memorylocations[0].name
        if alloc.kind == "ExternalInput":
            if name != partition_name:
                in_names.append(name)
        elif alloc.kind == "ExternalOutput":
            out_names.append(name)
            shape = tuple(alloc.tensor_shape)
            dtype = mybir.dt.np(alloc.dtype)
            out_avals.append(jax.core.ShapedArray(shape, dtype))
            zero_shapes.append((shape, dtype))
    n_params = len(in_names)
    n_outs = len(out_names)
    all_names = in_names + out_names
    if partition_name is not None:
        all_names = all_names + [partition_name]
    donate = tuple(range(n_params, n_params + n_outs))

    def _body(*args):
        operands = list(args)
        if partition_name is not None:
            operands.append(partition_id_tensor())
        outs = _bass_exec_p.bind(
            *operands,
            out_avals=tuple(out_avals),
            in_names=tuple(all_names),
            out_names=tuple(out_names),
            lowering_input_output_aliases=(),
            sim_require_finite=True,
            sim_require_nnan=True,
            nc=nc,
        )
        return tuple(outs)

    devices = jax.devices()[:NCORES]
    mesh = Mesh(np.asarray(devices), ("core",))
    in_specs = (PartitionSpec("core"),) * (n_params + n_outs)
    out_specs = (PartitionSpec("core"),) * n_outs
    sharded = jax.jit(
        shard_map(_body, mesh=mesh, in_specs=in_specs, out_specs=out_specs,
                  check_rep=False),
        donate_argnums=donate,
        keep_unused=True,
    )

    def run(in_maps):
        concat_in = [
            np.concatenate([np.asarray(m[name]) for m in in_maps], axis=0)
            for name in in_names
        ]
        concat_zeros = [
            np.zeros((NCORES * s[0], *s[1:]), d) for (s, d) in zero_shapes
        ]
        out_arrs = sharded(*concat_in, *concat_zeros)
        out_np = [np.asarray(a) for a in out_arrs]
        return [
            {
                name: out_np[i].reshape(NCORES, *out_avals[i].shape)[c]
                for i, name in enumerate(out_names)
            }
            for c in range(NCORES)
        ]

    return run


def get_runner(with_mask=False):
    key = ("runner", with_mask)
    if key not in _CACHE:
        nc = _build_nc(with_mask=with_mask)
        _CACHE[("nc", with_mask)] = nc
        _CACHE[key] = _make_runner(nc)
    return _CACHE[key]


def prep(input, features, features_mask, W, scale, attn_bias):
    """Host-side shard + layout prep -> (list of 8 per-core dicts, with_mask)."""
    input = np.asarray(input, np.float32)
    features = np.asarray(features, np.float32)
    mask = np.asarray(features_mask).reshape(B, T)
    W = np.asarray(W, np.float32)
    scale = np.asarray(scale, np.float32)
    attn_bias = np.asarray(attn_bias, np.float32)

    with_mask = not bool(mask.all())
    w = (scale.reshape(()) * W[0] / np.linalg.norm(W[0]))  # [S] f32
    # wq2[p, sb] = w[sb*PB + p]
    wq = np.ascontiguousarray(w.astype(BF16).reshape(SB, PB).T)
    q = input + attn_bias[None, :]                          # [B, S] f32

    f_bf = features.astype(BF16)                            # [B, T, S]

    in_maps = []
    for i in range(NCORES):
        bs = slice(i * BL, (i + 1) * BL)
        fc = f_bf[bs]                                       # [BL, T, S]
        ft = np.ascontiguousarray(fc.transpose(0, 2, 1)).reshape(BL, SB, PB, T)
        fn = fc.reshape(BL, PB, C, S)
        # qb2[p, sb*BL + b] = q[b, sb*PB + p]
        qb = np.ascontiguousarray(
            q[bs].reshape(BL, SB, PB).transpose(2, 1, 0).reshape(PB, SB * BL))
        m = {
            "ft": np.ascontiguousarray(ft),
            "fn": np.ascontiguousarray(fn),
            "qb": qb,
            "wq": wq,
        }
        if with_mask:
            m["mk"] = np.ascontiguousarray(
                mask[bs].astype(np.float32).reshape(BL, 1, T))
        in_maps.append(m)
    return in_maps, with_mask


def run_device(in_maps, with_mask=False):
    results = get_runner(with_mask=with_mask)(in_maps)
    context = np.concatenate(
        [r["ctx"].reshape(BL, S) for r in results], axis=0).astype(np.float32)
    weights = np.concatenate(
        [r["wts"].reshape(BL, T) for r in results], axis=0).astype(np.float32)
    return context, weights[:, :, None]


def kernel(input, features, features_mask, W, scale, attn_bias):
    in_maps, with_mask = prep(input, features, features_mask, W, scale, attn_bias)
    context, weights = run_device(in_maps, with_mask=with_mask)
    return (context, weights)
